# revision 38
# baseline (speedup 1.0000x reference)
"""AugmentedLSTMCell on 8 TRN2 NeuronCores — data-parallel over batch.

Layout: feature-on-partition (transposed). Per core: B_loc=2048 batch rows.
  proj.T[j, b] = sum_e W[j, e] * in[b, e]
  psum [128j, 2048b] accumulates Wi-proj + Ws-proj k-tiles
  (the "fused = proj_in + proj_st" add comes free via PSUM accumulation).
  ScalarE applies per-feature bias + sigmoid/tanh straight out of PSUM.

Mixed precision per gate block (i,f,m,o,hw,hwp):
  - sigmoid gates (i,f,o,hw): fp8-e4m3 weights+activations with
    perf_mode=DoubleRow (256-deep contraction per matmul, ~1.7x PE rate).
    Weights pre-scaled x64 to clear the e4m3 subnormal range; compensated
    by scale=1/64 in the ScalarE activation.
  - tanh block (m) and the linear passthrough (hwp): bf16 (error-critical
    paths; fp8 there pushes rel_err past the 2e-2 gate).
Host transposes outputs back to [B, H].
"""
import sys
import types

sys.path.insert(0, "/opt/trn_rl_repo")
sys.path.insert(0, "/root/.axon_site")

# Shim antenv.axon_hooks (missing on this image) so trace=True can profile.
if "antenv.axon_hooks" not in sys.modules:
    _hooks = types.ModuleType("antenv.axon_hooks")
    _state = {"hook": None}
    _hooks.set_axon_ntff_profile_hook = lambda h: _state.__setitem__("hook", h)
    _hooks.get_axon_ntff_profile_hook = lambda: _state["hook"]
    sys.modules["antenv.axon_hooks"] = _hooks
    try:
        from trn_agent_boot.trn_boot import _ntff_profile_via_ctypes

        _hooks.set_axon_ntff_profile_hook(
            _ntff_profile_via_ctypes("/opt/axon/libaxon_pjrt.so")
        )
    except Exception:
        pass

import numpy as np
import ml_dtypes

import concourse.bass as bass
import concourse.bacc as bacc
import concourse.mybir as mybir
from concourse import tile
from concourse.bass_utils import run_bass_kernel_spmd

BF16 = ml_dtypes.bfloat16
F8 = ml_dtypes.float8_e4m3fn

N_CORES = 8
B, E, H = 16384, 1024, 1024
BL = B // N_CORES          # 2048 batch rows per core
KT = E // 128              # 8 contraction k-tiles
KP = KT // 2               # 4 DoubleRow k-pairs
NJI = 6 * H // 128         # 48 feature tiles of proj_in
NJS = 5 * H // 128         # 40 feature tiles of proj_st
NT = H // 128              # 8 H-slices
BC = 512                   # matmul moving free dim (one PSUM bank)
NBC = BL // BC             # batch chunks per matmul group
WS = 64.0                  # weight pre-scale (all blocks; undone in act)

# Per-block dtype: '8' = fp8 DoubleRow, 'b' = bf16.
# Blocks: 0=i, 1=f, 2=m(tanh), 3=o, 4=hw, 5=hw_proj(linear)
CFG_WI = ["8", "8", "b", "8", "8", "b"]
CFG_WS = ["8", "8", "b", "8", "8"]
# m (tanh) block: 6 of 8 k-tiles in bf16, last k-pair fp8-DR. Cuts 13.6us
# of PE time; raises rel_err to ~1.7e-2 (sim+HW verified) vs the 2e-2 gate.
M_NB = 6

AF = mybir.ActivationFunctionType
DR = mybir.MatmulPerfMode.DoubleRow


def build_nc():
    nc = bacc.Bacc(None, target_bir_lowering=False)
    f32, bf16, f8 = mybir.dt.float32, mybir.dt.bfloat16, mybir.dt.float8e4

    xb_d = nc.declare_dram_parameter("xb", [128, KT, BL], bf16, isOutput=False)
    hb_d = nc.declare_dram_parameter("hb", [128, KT, BL], bf16, isOutput=False)
    x8_d = nc.declare_dram_parameter("x8", [128, KT, BL], f8, isOutput=False)
    h8_d = nc.declare_dram_parameter("h8", [128, KT, BL], f8, isOutput=False)
    cT = nc.declare_dram_parameter("cT", [H, BL], bf16, isOutput=False)
    wib = nc.declare_dram_parameter("wib", [NJI, 128, E], bf16, isOutput=False)
    wsb = nc.declare_dram_parameter("wsb", [NJS, 128, H], bf16, isOutput=False)
    wi8 = nc.declare_dram_parameter("wi8", [NJI, 128, KT, 128], f8, isOutput=False)
    ws8 = nc.declare_dram_parameter("ws8", [NJS, 128, KT, 128], f8, isOutput=False)
    bias = nc.declare_dram_parameter("bias", [128, NJI], f32, isOutput=False)
    outT = nc.declare_dram_parameter("outT", [H, BL], bf16, isOutput=True)
    memT = nc.declare_dram_parameter("memT", [H, BL], bf16, isOutput=True)

    with tile.TileContext(nc) as tc:
        with (
            tc.tile_pool(name="resident", bufs=1) as resident,
            tc.tile_pool(name="wpool8", bufs=12) as wpool8,
            tc.tile_pool(name="wpoolb", bufs=4) as wpoolb,
            tc.tile_pool(name="cpool", bufs=2) as cpool,
            tc.tile_pool(name="psum", bufs=2, space="PSUM") as psum_pool,
            tc.tile_pool(name="gates", bufs=10) as gate_pool,
            tc.tile_pool(name="tmp", bufs=5) as tmp_pool,
            tc.tile_pool(name="outp", bufs=2) as out_pool,
            tc.tile_pool(name="outc", bufs=6) as outc_pool,
        ):
            # ---- resident tiles -------------------------------------------
            bias_sb = resident.tile([128, NJI], f32, tag="bias")

            # bf16 x/h as single [128p, KT, BL] tiles (few coarse DMAs).
            xts = resident.tile([128, KT, BL], bf16, tag="xt", name="xt")
            hts = resident.tile([128, KT, BL], bf16, tag="ht", name="ht")

            def rhs_x(k, bc):
                return xts[:, k, bc * BC : (bc + 1) * BC]

            def rhs_h(k, bc):
                return hts[:, k, bc * BC : (bc + 1) * BC]

            # fp8 x/h, [128p, KT, BL] so a [:, 2kp:2kp+2, cols] slice is the
            # 3D DoubleRow rhs AP.
            x8s = resident.tile([128, KT, BL], f8, tag="x8", name="x8")
            h8s = resident.tile([128, KT, BL], f8, tag="h8", name="h8")

            # ---- startup DMA in strict need-order -------------------------
            # Each dma_start costs ~0.6us of descriptor-issue time on its
            # engine ring, and a ring with many large transfers queued
            # BLOCKS that engine's later instructions (backpressure) — so
            # the scalar (activation) ring gets only 6 small weight
            # preloads, gpsimd carries the bulk, sync gets h8 + the
            # in-loop weight stream.
            # The DMA system fair-shares bandwidth across ALL outstanding
            # transfers, so only the fp8 residents (4MB) are issued up
            # front; the 8MB bf16 bulk is issued later from the scalar
            # ring, naturally gated behind the first activations.
            w_i0 = wpool8.tile([128, KT, 128], f8, tag="w8")
            w_s0 = wpool8.tile([128, KT, 128], f8, tag="w8")
            w_i1 = wpool8.tile([128, KT, 128], f8, tag="w8")
            w_s1 = wpool8.tile([128, KT, 128], f8, tag="w8")
            # the very first matmul gates on just 128KB (x8 kp0 bc0) +
            # 32KB (w_i0 kp0); i0's Ws phase gates on h8 kp0 only
            nc.scalar.dma_start(w_i0[:, 0:2, :], wi8[0][:, 0:2, :])
            nc.gpsimd.dma_start(x8s[:, 0:2, :BC], x8_d[:, 0:2, :BC])
            nc.scalar.dma_start(w_i0[:, 2:8, :], wi8[0][:, 2:8, :])
            nc.gpsimd.dma_start(x8s[:, 0:2, BC:], x8_d[:, 0:2, BC:])
            nc.gpsimd.dma_start(x8s[:, 2:8, :], x8_d[:, 2:8, :])
            nc.scalar.dma_start(w_s0[:], ws8[0])
            # i1's weights preloaded on the otherwise-idle scalar ring so the
            # t0->t1 boundary doesn't wait on the (in-order, WAR-gated) sync
            # ring weight stream.
            nc.scalar.dma_start(w_i1[:], wi8[1])
            nc.scalar.dma_start(w_s1[:], ws8[1])
            nc.gpsimd.dma_start(h8s[:, 0:2, :], h8_d[:, 0:2, :])
            nc.gpsimd.dma_start(h8s[:, 2:8, :], h8_d[:, 2:8, :])
            nc.sync.dma_start(bias_sb[:], bias[:])

            # ---- one gate feature-tile ------------------------------------
            # nb = number of bf16 k-tiles (from k=0); the remaining
            # (KT-nb)/2 k-pairs run as fp8 DoubleRow. nb=0 -> pure fp8,
            # nb=KT -> pure bf16. Error of the block scales ~sqrt((KT-nb)/KT)
            # of the pure-fp8 error.
            def feature_tile(jt, func, dt8, w_i=None, w_s=None, chunk_act=1,
                             bc0=0, bc1=NBC, nb=None):
                """proj tile [128j, (bc1-bc0)*BC] -> activated gate (bf16)."""
                if nb is None:
                    nb = 0 if dt8 else KT
                has_st = jt < NJS
                w8_i = w8_s = None
                if nb > 0:
                    if w_i is None:
                        w_i = wpoolb.tile([128, E], bf16, tag="wb")
                        nc.sync.dma_start(w_i[:, : nb * 128], wib[jt][:, : nb * 128])
                    if has_st and w_s is None:
                        w_s = wpoolb.tile([128, H], bf16, tag="wb")
                        nc.sync.dma_start(w_s[:, : nb * 128], wsb[jt][:, : nb * 128])
                else:
                    w8_i, w8_s = w_i, w_s
                if nb < KT:
                    if w8_i is None:
                        w8_i = wpool8.tile([128, KT, 128], f8, tag="w8")
                        nc.sync.dma_start(w8_i[:, nb:KT, :], wi8[jt][:, nb:KT, :])
                    if has_st and w8_s is None:
                        w8_s = wpool8.tile([128, KT, 128], f8, tag="w8")
                        nc.sync.dma_start(w8_s[:, nb:KT, :], ws8[jt][:, nb:KT, :])
                width = (bc1 - bc0) * BC
                ps = psum_pool.tile([128, width], f32, tag="ps")

                def side(wb_t, w8_t, rhs_b, rhs8, first, last):
                    for k in range(nb):
                        lhsT = wb_t[:, k * 128 : (k + 1) * 128]
                        for bc in range(bc0, bc1):
                            lo = (bc - bc0) * BC
                            nc.tensor.matmul(
                                ps[:, lo : lo + BC], lhsT, rhs_b(k, bc),
                                start=(first and k == 0),
                                stop=(last and nb == KT and k == KT - 1),
                            )
                    for kp in range(nb // 2, KP):
                        lhsT = w8_t[:, 2 * kp : 2 * kp + 2, :]
                        for bc in range(bc0, bc1):
                            lo = (bc - bc0) * BC
                            nc.tensor.matmul(
                                ps[:, lo : lo + BC], lhsT,
                                rhs8[:, 2 * kp : 2 * kp + 2, bc * BC : (bc + 1) * BC],
                                start=(first and nb == 0 and kp == 0),
                                stop=(last and kp == KP - 1),
                                perf_mode=DR,
                            )

                side(w_i, w8_i, rhs_x, x8s, True, not has_st)
                if has_st:
                    side(w_s, w8_s, rhs_h, h8s, False, True)
                g = gate_pool.tile([128, width], bf16, tag="g")
                cw = width // chunk_act
                for a in range(chunk_act):
                    sl = slice(a * cw, (a + 1) * cw)
                    nc.scalar.activation(
                        g[:, sl], ps[:, sl], func,
                        bias=bias_sb[:, jt : jt + 1], scale=1.0 / WS,
                    )
                return g

            mult, addop, subop = (
                mybir.AluOpType.mult,
                mybir.AluOpType.add,
                mybir.AluOpType.subtract,
            )

            def load_ct(t):
                ct = cpool.tile([128, BL], bf16, tag="c")
                nc.sync.dma_start(ct[:], cT[t * 128 : (t + 1) * 128, :])
                return ct

            def ew_pre(t, i_g, m_g, f_g, ct):
                t1 = tmp_pool.tile([128, BL], bf16, tag="tmp")
                nc.vector.tensor_tensor(t1[:], i_g[:], m_g[:], mult)
                t2 = tmp_pool.tile([128, BL], bf16, tag="tmp")
                nc.vector.tensor_tensor(t2[:], f_g[:], ct[:], mult)
                mem = out_pool.tile([128, BL], bf16, tag="mem")
                nc.vector.tensor_tensor(mem[:], t1[:], t2[:], addop)
                nc.gpsimd.dma_start(memT[t * 128 : (t + 1) * 128, :], mem[:])
                tmem = tmp_pool.tile([128, BL], bf16, tag="tmp")
                nc.scalar.activation(tmem[:], mem[:], AF.Tanh)
                return tmem

            def ew_post(o_g, tmem, hwp):
                outp = tmp_pool.tile([128, BL], bf16, tag="tmp")
                nc.vector.tensor_tensor(outp[:], o_g[:], tmem[:], mult)
                u = tmp_pool.tile([128, BL], bf16, tag="tmp")
                nc.vector.tensor_tensor(u[:], outp[:], hwp[:], subop)
                return u

            def blend(t, hw_tile, u, hwp, col0, ncols, nchunk):
                # out[:, col0:col0+ncols] = hwp + hw*u over `nchunk` pieces
                ec = ncols // nchunk
                for e in range(nchunk):
                    sl = slice(col0 + e * ec, col0 + (e + 1) * ec)
                    lsl = slice(e * ec, (e + 1) * ec)
                    v = tmp_pool.tile([128, ec], bf16, tag="v")
                    nc.vector.tensor_tensor(v[:], hw_tile[:, lsl], u[:, sl], mult)
                    outf = outc_pool.tile([128, ec], bf16, tag="out")
                    nc.vector.tensor_tensor(outf[:], v[:], hwp[:, sl], addop)
                    # last group: alternate rings so the final chunks' issue
                    # (~0.65us each) and drain parallelize
                    eng = nc.scalar if (t == NT - 1 and e % 2) else nc.gpsimd
                    eng.dma_start(outT[t * 128 : (t + 1) * 128, sl], outf[:])

            def ft(blk, t, func, **kw):
                if blk == 2 and "nb" not in kw:
                    kw["nb"] = M_NB
                return feature_tile(blk * NT + t, func, CFG_WI[blk] == "8", **kw)

            # ---- PE warmup ------------------------------------------------
            # The tensor engine ramps its clock with sustained use (first
            # ~16 matmuls otherwise run at 426-585ns instead of 213ns).
            # Spend the DMA dead-zone (~6.5-13us, before the first real
            # operands land) on dummy matmuls over a zeroed scratch tile so
            # the real stream starts at full clock.
            scratch = resident.tile([128, BC], bf16, tag="warm")
            nc.vector.memset(scratch[:], 0.0)
            wps = psum_pool.tile([128, BC], f32, tag="ps")
            for _ in range(12):
                nc.tensor.matmul(
                    wps[:], scratch[:, 0:128], scratch[:], start=True, stop=True
                )

            # ---- t=0/t=1 software-pipelined prologue ----------------------
            # All eight fp8 tiles of t0+t1 run first (they only need the 4MB
            # of fp8 residents); the four bf16 tiles are deferred until the
            # 8MB of bf16 residents have streamed in (~54us of PE cover).
            i0 = ft(0, 0, AF.Sigmoid, w_i=w_i0, w_s=w_s0)
            # bf16 bulk on the gpsimd ring, but throttled: a tiny Vector op
            # reading i0's gate writes into hts/xts first, so the bulk DMA
            # (WAR on that region) can't start until i0's activation is
            # done — it would otherwise fair-share DMA bandwidth away from
            # the fp8 residents feeding the first tiles. (m0 needs hts by
            # ~54us of PE time, hwp0 needs xts by ~68us.)
            nc.vector.tensor_tensor(hts[:, 0, 0:8], i0[:, 0:8], i0[:, 0:8], mult)
            nc.vector.tensor_tensor(xts[:, 0, 0:8], i0[:, 0:8], i0[:, 0:8], mult)
            nc.gpsimd.dma_start(hts[:, 0:4, :], hb_d[:, 0:4, :])
            nc.gpsimd.dma_start(hts[:, 4:8, :], hb_d[:, 4:8, :])
            f0 = ft(1, 0, AF.Sigmoid)
            nc.gpsimd.dma_start(xts[:, 0:4, :], xb_d[:, 0:4, :])
            nc.gpsimd.dma_start(xts[:, 4:8, :], xb_d[:, 4:8, :])
            o0 = ft(3, 0, AF.Sigmoid)
            hw0 = ft(4, 0, AF.Sigmoid, chunk_act=4)
            i1 = ft(0, 1, AF.Sigmoid, w_i=w_i1, w_s=w_s1)
            f1 = ft(1, 1, AF.Sigmoid)
            o1 = ft(3, 1, AF.Sigmoid)
            ct0 = load_ct(0)
            ct1 = load_ct(1)
            hw1 = ft(4, 1, AF.Sigmoid, chunk_act=4)
            m0 = ft(2, 0, AF.Tanh)
            hwp0 = ft(5, 0, AF.Identity)
            tmem0 = ew_pre(0, i0, m0, f0, ct0)
            u0 = ew_post(o0, tmem0, hwp0)
            blend(0, hw0, u0, hwp0, 0, BL, 4)
            m1 = ft(2, 1, AF.Tanh)
            hwp1 = ft(5, 1, AF.Identity)
            tmem1 = ew_pre(1, i1, m1, f1, ct1)
            u1 = ew_post(o1, tmem1, hwp1)
            blend(1, hw1, u1, hwp1, 0, BL, 4)

            # ---- steady state ---------------------------------------------
            for t in range(2, NT):
                i_g = ft(0, t, AF.Sigmoid)
                ct = load_ct(t)
                m_g = ft(2, t, AF.Tanh)
                f_g = ft(1, t, AF.Sigmoid)
                tmem = ew_pre(t, i_g, m_g, f_g, ct)
                o_g = ft(3, t, AF.Sigmoid)
                hwp = ft(5, t, AF.Identity)
                u = ew_post(o_g, tmem, hwp)
                if t < NT - 1:
                    hw_g = ft(4, t, AF.Sigmoid, chunk_act=4)
                    blend(t, hw_g, u, hwp, 0, BL, 4)
                else:
                    # Last group: hw in bc-halves, finely chunked blends, so
                    # the tail after the final matmuls is just one small
                    # blend + DMA.
                    for half in range(2):
                        hw_h = ft(4, t, AF.Sigmoid, chunk_act=4,
                                  bc0=2 * half, bc1=2 * half + 2)
                        blend(t, hw_h, u, hwp, half * (BL // 2), BL // 2, 4)

    nc.compile()
    return nc


_NC_CACHE = None


def _get_nc():
    global _NC_CACHE
    if _NC_CACHE is None:
        _NC_CACHE = build_nc()
    return _NC_CACHE


def _pack_weights_bf16(W, njt):
    # W [njt*128 j, K e] -> [njt, 128 p, K] with [jt, p, k*128+m] = W[jt*128+m, k*128+p]
    K = W.shape[1]
    kt = K // 128
    return np.ascontiguousarray(
        (W * WS).reshape(njt, 128, kt, 128).transpose(0, 3, 2, 1).reshape(njt, 128, K)
    ).astype(BF16)


def _pack_weights_f8(W, njt):
    # W [njt*128 j, K e] -> [njt, 128 p, kt, 128 m] = W[jt*128+m, k*128+p]*WS
    K = W.shape[1]
    kt = K // 128
    return np.ascontiguousarray(
        (W * WS).reshape(njt, 128, kt, 128).transpose(0, 3, 2, 1)
    ).astype(F8)


def _pack_act(aT, dt):
    # aT [K, BL] -> [128 p, kt, BL] with [p, k, b] = aT[k*128+p, b]
    K = aT.shape[0]
    kt = K // 128
    return np.ascontiguousarray(aT.reshape(kt, 128, BL).transpose(1, 0, 2)).astype(dt)


def prepare_in_maps(x, h, c, Wi, bi, Ws, bs):
    Wi = np.asarray(Wi, np.float32)
    Ws = np.asarray(Ws, np.float32)
    wib_p = _pack_weights_bf16(Wi, NJI)
    wsb_p = _pack_weights_bf16(Ws, NJS)
    wi8_p = _pack_weights_f8(Wi, NJI)
    ws8_p = _pack_weights_f8(Ws, NJS)
    bias_comb = np.concatenate(
        [np.asarray(bi[: 5 * H], np.float32) + np.asarray(bs, np.float32),
         np.asarray(bi[5 * H :], np.float32)]
    )
    bias_pack = np.ascontiguousarray(bias_comb.reshape(NJI, 128).T).astype(np.float32)

    in_maps = []
    for i in range(N_CORES):
        s = slice(i * BL, (i + 1) * BL)
        xT = np.ascontiguousarray(np.asarray(x[s], np.float32).T)
        hT = np.ascontiguousarray(np.asarray(h[s], np.float32).T)
        in_maps.append(
            {
                "xb": _pack_act(xT, BF16),
                "hb": _pack_act(hT, BF16),
                "x8": _pack_act(xT, F8),
                "h8": _pack_act(hT, F8),
                "cT": np.ascontiguousarray(np.asarray(c[s], np.float32).T).astype(BF16),
                "wib": wib_p,
                "wsb": wsb_p,
                "wi8": wi8_p,
                "ws8": ws8_p,
                "bias": bias_pack,
            }
        )
    return in_maps


def run(in_maps, trace=False):
    nc = _get_nc()
    res = run_bass_kernel_spmd(nc, in_maps, core_ids=list(range(N_CORES)), trace=trace)
    out = np.empty((B, H), np.float32)
    mem = np.empty((B, H), np.float32)
    for i in range(N_CORES):
        s = slice(i * BL, (i + 1) * BL)
        out[s] = res.results[i]["outT"].T.astype(np.float32)
        mem[s] = res.results[i]["memT"].T.astype(np.float32)
    return (out, mem), res


def kernel(x, h, c, Wi, bi, Ws, bs):
    in_maps = prepare_in_maps(x, h, c, Wi, bi, Ws, bs)
    (out, mem), _ = run(in_maps, trace=False)
    return out, mem


# revision 40
# speedup vs baseline: 1.0024x; 1.0024x over previous
"""AugmentedLSTMCell on 8 TRN2 NeuronCores — data-parallel over batch.

Layout: feature-on-partition (transposed). Per core: B_loc=2048 batch rows.
  proj.T[j, b] = sum_e W[j, e] * in[b, e]
  psum [128j, 2048b] accumulates Wi-proj + Ws-proj k-tiles
  (the "fused = proj_in + proj_st" add comes free via PSUM accumulation).
  ScalarE applies per-feature bias + sigmoid/tanh straight out of PSUM.

Mixed precision per gate block (i,f,m,o,hw,hwp):
  - sigmoid gates (i,f,o,hw): fp8-e4m3 weights+activations with
    perf_mode=DoubleRow (256-deep contraction per matmul, ~1.7x PE rate).
    Weights pre-scaled x64 to clear the e4m3 subnormal range; compensated
    by scale=1/64 in the ScalarE activation.
  - tanh block (m) and the linear passthrough (hwp): bf16 (error-critical
    paths; fp8 there pushes rel_err past the 2e-2 gate).
Host transposes outputs back to [B, H].
"""
import sys
import types

sys.path.insert(0, "/opt/trn_rl_repo")
sys.path.insert(0, "/root/.axon_site")

# Shim antenv.axon_hooks (missing on this image) so trace=True can profile.
if "antenv.axon_hooks" not in sys.modules:
    _hooks = types.ModuleType("antenv.axon_hooks")
    _state = {"hook": None}
    _hooks.set_axon_ntff_profile_hook = lambda h: _state.__setitem__("hook", h)
    _hooks.get_axon_ntff_profile_hook = lambda: _state["hook"]
    sys.modules["antenv.axon_hooks"] = _hooks
    try:
        from trn_agent_boot.trn_boot import _ntff_profile_via_ctypes

        _hooks.set_axon_ntff_profile_hook(
            _ntff_profile_via_ctypes("/opt/axon/libaxon_pjrt.so")
        )
    except Exception:
        pass

import numpy as np
import ml_dtypes

import concourse.bass as bass
import concourse.bacc as bacc
import concourse.mybir as mybir
from concourse import tile
from concourse.bass_utils import run_bass_kernel_spmd

BF16 = ml_dtypes.bfloat16
F8 = ml_dtypes.float8_e4m3fn

N_CORES = 8
B, E, H = 16384, 1024, 1024
BL = B // N_CORES          # 2048 batch rows per core
KT = E // 128              # 8 contraction k-tiles
KP = KT // 2               # 4 DoubleRow k-pairs
NJI = 6 * H // 128         # 48 feature tiles of proj_in
NJS = 5 * H // 128         # 40 feature tiles of proj_st
NT = H // 128              # 8 H-slices
BC = 512                   # matmul moving free dim (one PSUM bank)
NBC = BL // BC             # batch chunks per matmul group
WS = 64.0                  # weight pre-scale (all blocks; undone in act)

# Per-block dtype: '8' = fp8 DoubleRow, 'b' = bf16.
# Blocks: 0=i, 1=f, 2=m(tanh), 3=o, 4=hw, 5=hw_proj(linear)
CFG_WI = ["8", "8", "b", "8", "8", "b"]
CFG_WS = ["8", "8", "b", "8", "8"]
# m (tanh) block: 6 of 8 k-tiles in bf16, last k-pair fp8-DR. Cuts 13.6us
# of PE time; raises rel_err to ~1.7e-2 (sim+HW verified) vs the 2e-2 gate.
M_NB = 6

AF = mybir.ActivationFunctionType
DR = mybir.MatmulPerfMode.DoubleRow


def build_nc():
    nc = bacc.Bacc(None, target_bir_lowering=False)
    f32, bf16, f8 = mybir.dt.float32, mybir.dt.bfloat16, mybir.dt.float8e4

    xb_d = nc.declare_dram_parameter("xb", [128, KT, BL], bf16, isOutput=False)
    hb_d = nc.declare_dram_parameter("hb", [128, KT, BL], bf16, isOutput=False)
    x8_d = nc.declare_dram_parameter("x8", [128, KT, BL], f8, isOutput=False)
    h8_d = nc.declare_dram_parameter("h8", [128, KT, BL], f8, isOutput=False)
    cT = nc.declare_dram_parameter("cT", [H, BL], bf16, isOutput=False)
    wib = nc.declare_dram_parameter("wib", [NJI, 128, E], bf16, isOutput=False)
    wsb = nc.declare_dram_parameter("wsb", [NJS, 128, H], bf16, isOutput=False)
    wi8 = nc.declare_dram_parameter("wi8", [NJI, 128, KT, 128], f8, isOutput=False)
    ws8 = nc.declare_dram_parameter("ws8", [NJS, 128, KT, 128], f8, isOutput=False)
    bias = nc.declare_dram_parameter("bias", [128, NJI], f32, isOutput=False)
    outT = nc.declare_dram_parameter("outT", [H, BL], bf16, isOutput=True)
    memT = nc.declare_dram_parameter("memT", [H, BL], bf16, isOutput=True)

    with tile.TileContext(nc) as tc:
        with (
            tc.tile_pool(name="resident", bufs=1) as resident,
            tc.tile_pool(name="wpool8", bufs=12) as wpool8,
            tc.tile_pool(name="wpoolb", bufs=4) as wpoolb,
            tc.tile_pool(name="cpool", bufs=2) as cpool,
            tc.tile_pool(name="psum", bufs=2, space="PSUM") as psum_pool,
            tc.tile_pool(name="gates", bufs=10) as gate_pool,
            tc.tile_pool(name="tmp", bufs=5) as tmp_pool,
            tc.tile_pool(name="outp", bufs=2) as out_pool,
            tc.tile_pool(name="outc", bufs=6) as outc_pool,
        ):
            # ---- resident tiles -------------------------------------------
            bias_sb = resident.tile([128, NJI], f32, tag="bias")

            # bf16 x/h as single [128p, KT, BL] tiles (few coarse DMAs).
            xts = resident.tile([128, KT, BL], bf16, tag="xt", name="xt")
            hts = resident.tile([128, KT, BL], bf16, tag="ht", name="ht")

            def rhs_x(k, bc):
                return xts[:, k, bc * BC : (bc + 1) * BC]

            def rhs_h(k, bc):
                return hts[:, k, bc * BC : (bc + 1) * BC]

            # fp8 x/h, [128p, KT, BL] so a [:, 2kp:2kp+2, cols] slice is the
            # 3D DoubleRow rhs AP.
            x8s = resident.tile([128, KT, BL], f8, tag="x8", name="x8")
            h8s = resident.tile([128, KT, BL], f8, tag="h8", name="h8")

            # ---- startup DMA in strict need-order -------------------------
            # Each dma_start costs ~0.6us of descriptor-issue time on its
            # engine ring, and a ring with many large transfers queued
            # BLOCKS that engine's later instructions (backpressure) — so
            # the scalar (activation) ring gets only 6 small weight
            # preloads, gpsimd carries the bulk, sync gets h8 + the
            # in-loop weight stream.
            # The DMA system fair-shares bandwidth across ALL outstanding
            # transfers, so only the fp8 residents (4MB) are issued up
            # front; the 8MB bf16 bulk is issued later from the scalar
            # ring, naturally gated behind the first activations.
            w_i0 = wpool8.tile([128, KT, 128], f8, tag="w8")
            w_s0 = wpool8.tile([128, KT, 128], f8, tag="w8")
            w_i1 = wpool8.tile([128, KT, 128], f8, tag="w8")
            w_s1 = wpool8.tile([128, KT, 128], f8, tag="w8")
            # the very first matmul gates on just 128KB (x8 kp0 bc0) +
            # 32KB (w_i0 kp0); i0's Ws phase gates on h8 kp0 only
            nc.scalar.dma_start(w_i0[:, 0:2, :], wi8[0][:, 0:2, :])
            nc.gpsimd.dma_start(x8s[:, 0:2, :BC], x8_d[:, 0:2, :BC])
            nc.scalar.dma_start(w_i0[:, 2:8, :], wi8[0][:, 2:8, :])
            nc.gpsimd.dma_start(x8s[:, 0:2, BC:], x8_d[:, 0:2, BC:])
            nc.gpsimd.dma_start(x8s[:, 2:8, :], x8_d[:, 2:8, :])
            nc.scalar.dma_start(w_s0[:], ws8[0])
            # i1's weights preloaded on the otherwise-idle scalar ring so the
            # t0->t1 boundary doesn't wait on the (in-order, WAR-gated) sync
            # ring weight stream.
            nc.scalar.dma_start(w_i1[:], wi8[1])
            nc.scalar.dma_start(w_s1[:], ws8[1])
            nc.gpsimd.dma_start(h8s[:, 0:2, :], h8_d[:, 0:2, :])
            nc.gpsimd.dma_start(h8s[:, 2:8, :], h8_d[:, 2:8, :])
            nc.sync.dma_start(bias_sb[:], bias[:])

            # ---- one gate feature-tile ------------------------------------
            # nb = number of bf16 k-tiles (from k=0); the remaining
            # (KT-nb)/2 k-pairs run as fp8 DoubleRow. nb=0 -> pure fp8,
            # nb=KT -> pure bf16. Error of the block scales ~sqrt((KT-nb)/KT)
            # of the pure-fp8 error.
            def feature_tile(jt, func, dt8, w_i=None, w_s=None, chunk_act=1,
                             bc0=0, bc1=NBC, nb=None):
                """proj tile [128j, (bc1-bc0)*BC] -> activated gate (bf16)."""
                if nb is None:
                    nb = 0 if dt8 else KT
                has_st = jt < NJS
                w8_i = w8_s = None
                if nb > 0:
                    if w_i is None:
                        w_i = wpoolb.tile([128, E], bf16, tag="wb")
                        nc.sync.dma_start(w_i[:, : nb * 128], wib[jt][:, : nb * 128])
                    if has_st and w_s is None:
                        w_s = wpoolb.tile([128, H], bf16, tag="wb")
                        nc.sync.dma_start(w_s[:, : nb * 128], wsb[jt][:, : nb * 128])
                else:
                    w8_i, w8_s = w_i, w_s
                if nb < KT:
                    if w8_i is None:
                        w8_i = wpool8.tile([128, KT, 128], f8, tag="w8")
                        nc.sync.dma_start(w8_i[:, nb:KT, :], wi8[jt][:, nb:KT, :])
                    if has_st and w8_s is None:
                        w8_s = wpool8.tile([128, KT, 128], f8, tag="w8")
                        nc.sync.dma_start(w8_s[:, nb:KT, :], ws8[jt][:, nb:KT, :])
                width = (bc1 - bc0) * BC
                ps = psum_pool.tile([128, width], f32, tag="ps")

                def side(wb_t, w8_t, rhs_b, rhs8, first, last):
                    for k in range(nb):
                        lhsT = wb_t[:, k * 128 : (k + 1) * 128]
                        for bc in range(bc0, bc1):
                            lo = (bc - bc0) * BC
                            nc.tensor.matmul(
                                ps[:, lo : lo + BC], lhsT, rhs_b(k, bc),
                                start=(first and k == 0),
                                stop=(last and nb == KT and k == KT - 1),
                            )
                    for kp in range(nb // 2, KP):
                        lhsT = w8_t[:, 2 * kp : 2 * kp + 2, :]
                        for bc in range(bc0, bc1):
                            lo = (bc - bc0) * BC
                            nc.tensor.matmul(
                                ps[:, lo : lo + BC], lhsT,
                                rhs8[:, 2 * kp : 2 * kp + 2, bc * BC : (bc + 1) * BC],
                                start=(first and nb == 0 and kp == 0),
                                stop=(last and kp == KP - 1),
                                perf_mode=DR,
                            )

                side(w_i, w8_i, rhs_x, x8s, True, not has_st)
                if has_st:
                    side(w_s, w8_s, rhs_h, h8s, False, True)
                g = gate_pool.tile([128, width], bf16, tag="g")
                cw = width // chunk_act
                for a in range(chunk_act):
                    sl = slice(a * cw, (a + 1) * cw)
                    nc.scalar.activation(
                        g[:, sl], ps[:, sl], func,
                        bias=bias_sb[:, jt : jt + 1], scale=1.0 / WS,
                    )
                return g

            mult, addop, subop = (
                mybir.AluOpType.mult,
                mybir.AluOpType.add,
                mybir.AluOpType.subtract,
            )

            def load_ct(t):
                ct = cpool.tile([128, BL], bf16, tag="c")
                nc.sync.dma_start(ct[:], cT[t * 128 : (t + 1) * 128, :])
                return ct

            def ew_pre(t, i_g, m_g, f_g, ct):
                t1 = tmp_pool.tile([128, BL], bf16, tag="tmp")
                nc.vector.tensor_tensor(t1[:], i_g[:], m_g[:], mult)
                t2 = tmp_pool.tile([128, BL], bf16, tag="tmp")
                nc.vector.tensor_tensor(t2[:], f_g[:], ct[:], mult)
                mem = out_pool.tile([128, BL], bf16, tag="mem")
                nc.vector.tensor_tensor(mem[:], t1[:], t2[:], addop)
                nc.gpsimd.dma_start(memT[t * 128 : (t + 1) * 128, :], mem[:])
                tmem = tmp_pool.tile([128, BL], bf16, tag="tmp")
                nc.scalar.activation(tmem[:], mem[:], AF.Tanh)
                return tmem

            def ew_post(o_g, tmem, hwp):
                outp = tmp_pool.tile([128, BL], bf16, tag="tmp")
                nc.vector.tensor_tensor(outp[:], o_g[:], tmem[:], mult)
                u = tmp_pool.tile([128, BL], bf16, tag="tmp")
                nc.vector.tensor_tensor(u[:], outp[:], hwp[:], subop)
                return u

            def blend(t, hw_tile, u, hwp, col0, ncols, nchunk):
                # out[:, col0:col0+ncols] = hwp + hw*u over `nchunk` pieces
                ec = ncols // nchunk
                for e in range(nchunk):
                    sl = slice(col0 + e * ec, col0 + (e + 1) * ec)
                    lsl = slice(e * ec, (e + 1) * ec)
                    v = tmp_pool.tile([128, ec], bf16, tag="v")
                    nc.vector.tensor_tensor(v[:], hw_tile[:, lsl], u[:, sl], mult)
                    outf = outc_pool.tile([128, ec], bf16, tag="out")
                    nc.vector.tensor_tensor(outf[:], v[:], hwp[:, sl], addop)
                    # last group: alternate rings so the final chunks' issue
                    # (~0.65us each) and drain parallelize
                    eng = nc.scalar if (t == NT - 1 and e % 2) else nc.gpsimd
                    eng.dma_start(outT[t * 128 : (t + 1) * 128, sl], outf[:])

            def ft(blk, t, func, **kw):
                if blk == 2 and "nb" not in kw:
                    kw["nb"] = M_NB
                return feature_tile(blk * NT + t, func, CFG_WI[blk] == "8", **kw)

            # ---- PE warmup ------------------------------------------------
            # The tensor engine ramps its clock with sustained use (first
            # ~16 matmuls otherwise run at 426-585ns instead of 213ns).
            # Spend the DMA dead-zone (~6.3-13.3us, before the first real
            # operands land) on dummy matmuls over a zeroed scratch tile so
            # the real stream starts at full clock. 20 dummies (~7us at
            # ramping clock) hand off right at operand arrival.
            scratch = resident.tile([128, BC], bf16, tag="warm")
            nc.vector.memset(scratch[:], 0.0)
            wps = psum_pool.tile([128, BC], f32, tag="ps")
            for _ in range(20):
                nc.tensor.matmul(
                    wps[:], scratch[:, 0:128], scratch[:], start=True, stop=True
                )

            # ---- t=0/t=1 software-pipelined prologue ----------------------
            # All eight fp8 tiles of t0+t1 run first (they only need the 4MB
            # of fp8 residents); the four bf16 tiles are deferred until the
            # 8MB of bf16 residents have streamed in (~54us of PE cover).
            i0 = ft(0, 0, AF.Sigmoid, w_i=w_i0, w_s=w_s0)
            # bf16 bulk on the gpsimd ring, but throttled: a tiny Vector op
            # reading i0's gate writes into hts/xts first, so the bulk DMA
            # (WAR on that region) can't start until i0's activation is
            # done — it would otherwise fair-share DMA bandwidth away from
            # the fp8 residents feeding the first tiles. (m0 needs hts by
            # ~54us of PE time, hwp0 needs xts by ~68us.)
            nc.vector.tensor_tensor(hts[:, 0, 0:8], i0[:, 0:8], i0[:, 0:8], mult)
            nc.vector.tensor_tensor(xts[:, 0, 0:8], i0[:, 0:8], i0[:, 0:8], mult)
            nc.gpsimd.dma_start(hts[:, 0:4, :], hb_d[:, 0:4, :])
            nc.gpsimd.dma_start(hts[:, 4:8, :], hb_d[:, 4:8, :])
            f0 = ft(1, 0, AF.Sigmoid)
            nc.gpsimd.dma_start(xts[:, 0:4, :], xb_d[:, 0:4, :])
            nc.gpsimd.dma_start(xts[:, 4:8, :], xb_d[:, 4:8, :])
            o0 = ft(3, 0, AF.Sigmoid)
            hw0 = ft(4, 0, AF.Sigmoid, chunk_act=4)
            i1 = ft(0, 1, AF.Sigmoid, w_i=w_i1, w_s=w_s1)
            f1 = ft(1, 1, AF.Sigmoid)
            o1 = ft(3, 1, AF.Sigmoid)
            ct0 = load_ct(0)
            ct1 = load_ct(1)
            hw1 = ft(4, 1, AF.Sigmoid, chunk_act=4)
            m0 = ft(2, 0, AF.Tanh)
            hwp0 = ft(5, 0, AF.Identity)
            tmem0 = ew_pre(0, i0, m0, f0, ct0)
            u0 = ew_post(o0, tmem0, hwp0)
            blend(0, hw0, u0, hwp0, 0, BL, 4)
            m1 = ft(2, 1, AF.Tanh)
            hwp1 = ft(5, 1, AF.Identity)
            tmem1 = ew_pre(1, i1, m1, f1, ct1)
            u1 = ew_post(o1, tmem1, hwp1)
            blend(1, hw1, u1, hwp1, 0, BL, 4)

            # ---- steady state ---------------------------------------------
            for t in range(2, NT):
                i_g = ft(0, t, AF.Sigmoid)
                ct = load_ct(t)
                m_g = ft(2, t, AF.Tanh)
                f_g = ft(1, t, AF.Sigmoid)
                tmem = ew_pre(t, i_g, m_g, f_g, ct)
                o_g = ft(3, t, AF.Sigmoid)
                hwp = ft(5, t, AF.Identity)
                u = ew_post(o_g, tmem, hwp)
                if t < NT - 1:
                    hw_g = ft(4, t, AF.Sigmoid, chunk_act=4)
                    blend(t, hw_g, u, hwp, 0, BL, 4)
                else:
                    # Last group: hw in bc-halves, finely chunked blends, so
                    # the tail after the final matmuls is just one small
                    # blend + DMA.
                    for half in range(2):
                        hw_h = ft(4, t, AF.Sigmoid, chunk_act=4,
                                  bc0=2 * half, bc1=2 * half + 2)
                        blend(t, hw_h, u, hwp, half * (BL // 2), BL // 2, 4)

    nc.compile()
    return nc


_NC_CACHE = None


def _get_nc():
    global _NC_CACHE
    if _NC_CACHE is None:
        _NC_CACHE = build_nc()
    return _NC_CACHE


def _pack_weights_bf16(W, njt):
    # W [njt*128 j, K e] -> [njt, 128 p, K] with [jt, p, k*128+m] = W[jt*128+m, k*128+p]
    K = W.shape[1]
    kt = K // 128
    return np.ascontiguousarray(
        (W * WS).reshape(njt, 128, kt, 128).transpose(0, 3, 2, 1).reshape(njt, 128, K)
    ).astype(BF16)


def _pack_weights_f8(W, njt):
    # W [njt*128 j, K e] -> [njt, 128 p, kt, 128 m] = W[jt*128+m, k*128+p]*WS
    K = W.shape[1]
    kt = K // 128
    return np.ascontiguousarray(
        (W * WS).reshape(njt, 128, kt, 128).transpose(0, 3, 2, 1)
    ).astype(F8)


def _pack_act(aT, dt):
    # aT [K, BL] -> [128 p, kt, BL] with [p, k, b] = aT[k*128+p, b]
    K = aT.shape[0]
    kt = K // 128
    return np.ascontiguousarray(aT.reshape(kt, 128, BL).transpose(1, 0, 2)).astype(dt)


def prepare_in_maps(x, h, c, Wi, bi, Ws, bs):
    Wi = np.asarray(Wi, np.float32)
    Ws = np.asarray(Ws, np.float32)
    wib_p = _pack_weights_bf16(Wi, NJI)
    wsb_p = _pack_weights_bf16(Ws, NJS)
    wi8_p = _pack_weights_f8(Wi, NJI)
    ws8_p = _pack_weights_f8(Ws, NJS)
    bias_comb = np.concatenate(
        [np.asarray(bi[: 5 * H], np.float32) + np.asarray(bs, np.float32),
         np.asarray(bi[5 * H :], np.float32)]
    )
    bias_pack = np.ascontiguousarray(bias_comb.reshape(NJI, 128).T).astype(np.float32)

    in_maps = []
    for i in range(N_CORES):
        s = slice(i * BL, (i + 1) * BL)
        xT = np.ascontiguousarray(np.asarray(x[s], np.float32).T)
        hT = np.ascontiguousarray(np.asarray(h[s], np.float32).T)
        in_maps.append(
            {
                "xb": _pack_act(xT, BF16),
                "hb": _pack_act(hT, BF16),
                "x8": _pack_act(xT, F8),
                "h8": _pack_act(hT, F8),
                "cT": np.ascontiguousarray(np.asarray(c[s], np.float32).T).astype(BF16),
                "wib": wib_p,
                "wsb": wsb_p,
                "wi8": wi8_p,
                "ws8": ws8_p,
                "bias": bias_pack,
            }
        )
    return in_maps


def run(in_maps, trace=False):
    nc = _get_nc()
    res = run_bass_kernel_spmd(nc, in_maps, core_ids=list(range(N_CORES)), trace=trace)
    out = np.empty((B, H), np.float32)
    mem = np.empty((B, H), np.float32)
    for i in range(N_CORES):
        s = slice(i * BL, (i + 1) * BL)
        out[s] = res.results[i]["outT"].T.astype(np.float32)
        mem[s] = res.results[i]["memT"].T.astype(np.float32)
    return (out, mem), res


def kernel(x, h, c, Wi, bi, Ws, bs):
    in_maps = prepare_in_maps(x, h, c, Wi, bi, Ws, bs)
    (out, mem), _ = run(in_maps, trace=False)
    return out, mem
